# revision 12
# baseline (speedup 1.0000x reference)
"""DiffJPEG forward on 8 Trainium2 NeuronCores (Bass/Tile), data-parallel over batch.

Full inputs in, full outputs out. x: [64, 3, 512, 512] f32 -> [64, 3, 512, 512] f32.

Per-core pipeline (8 images/core), processed in 128x128 spatial chunks:
  p1 (PE):  contract H: RGB->(Y | Cb|Cr) color mix + DCT-H (+ chroma H-pool), out [W, Hf]
  p2 (PE):  contract W: DCT-W (+ chroma W-pool), out [Hf, Wf] = DCT coefs / strip batch
  mid (DVE/ACT): q = coef*invT (+DC bias); r = rint(q); y = (r + (q-r)^3)*T
  p3 (PE):  contract Hf: iDCT-H (+ chroma H-upsample), out [Wf, H]
  p4 (PE):  contract Wf: iDCT-W (+ chroma W-upsample) + YCbCr->RGB mix + /255
  out: Relu(x + 128/255) on ACT, min(x,1) on DVE, DMA out.

All DCT/color constants are folded into five families of 128-col matmul weights.
"""
import sys
import numpy as np

sys.path.insert(0, "/opt/trn_rl_repo")

from contextlib import ExitStack

import concourse.bass as bass
import concourse.tile as tile
from concourse import mybir
from concourse.bass_utils import run_bass_kernel_spmd

F32 = mybir.dt.float32
I32 = mybir.dt.int32

N_CORES = 8
IMGS_PER_CORE = 8
H = W = 512
NSTRIP = 4   # 128-row strips per image
NCH = 4      # 128-col chunks per strip


# ---------------------------------------------------------------------------
# constants
# ---------------------------------------------------------------------------
def _build_consts():
    """Returns (CONST [128, TOT] f32, dict name -> (col0, ncol))."""
    r = np.arange(8)
    COS = np.cos((2 * r[:, None] + 1) * r[None, :] * np.pi / 16).astype(np.float32)
    a = np.ones(8, dtype=np.float32)
    a[0] = 1.0 / np.sqrt(2.0)
    ka = (a * 0.5).astype(np.float32)

    M = np.array([[0.299, 0.587, 0.114],
                  [-0.168736, -0.331264, 0.5],
                  [0.5, -0.418688, -0.081312]], dtype=np.float32)
    Mi = np.array([[1.0, 0.0, 1.402],
                   [1.0, -0.344136, -0.714136],
                   [1.0, 1.772, 0.0]], dtype=np.float32)

    QUALITY = 80
    FACTOR = np.float32((5000.0 / QUALITY if QUALITY < 50 else 200.0 - QUALITY * 2.0) / 100.0)
    YT = np.array([
        [16, 11, 10, 16, 24, 40, 51, 61], [12, 12, 14, 19, 26, 58, 60, 55],
        [14, 13, 16, 24, 40, 57, 69, 56], [14, 17, 22, 29, 51, 87, 80, 62],
        [18, 22, 37, 56, 68, 109, 103, 77], [24, 35, 55, 64, 81, 104, 113, 92],
        [49, 64, 78, 87, 103, 121, 120, 101], [72, 92, 95, 98, 112, 100, 103, 99]],
        dtype=np.float32).T * FACTOR
    CT = np.full((8, 8), 99.0, dtype=np.float32)
    CT[:4, :4] = np.array([[17, 18, 24, 47], [18, 21, 26, 66],
                           [24, 26, 56, 99], [47, 66, 99, 99]], dtype=np.float32).T
    CT = CT * FACTOR

    mats = {}
    # p1 Y: [128 h, 128 hf], hf = u*16 + bi
    W1y = np.zeros((128, 128), dtype=np.float32)
    for hh in range(128):
        bi, x = hh // 8, hh % 8
        for uu in range(8):
            W1y[hh, uu * 16 + bi] = ka[uu] * COS[x, uu]
    for ci in range(3):
        mats[f"W1y{ci}"] = W1y * np.float32(255.0 * M[0, ci])
    # p1 C: [128 h, 128 cf], cf = cc*64 + u*8 + ub (H-pool folded)
    W1cb = np.zeros((128, 64), dtype=np.float32)
    for hh in range(128):
        rr = hh // 2
        ub, x = rr // 8, rr % 8
        for uu in range(8):
            W1cb[hh, uu * 8 + ub] = ka[uu] * COS[x, uu] * 0.5
    for ci in range(3):
        W1c = np.zeros((128, 128), dtype=np.float32)
        W1c[:, 0:64] = W1cb * np.float32(255.0 * M[1, ci])
        W1c[:, 64:128] = W1cb * np.float32(255.0 * M[2, ci])
        mats[f"W1c{ci}"] = W1c
    # p2 Y: [128 w, 128 wf], wf = v*16 + bj
    W2y = np.zeros((128, 128), dtype=np.float32)
    for ww in range(128):
        bj, y = ww // 8, ww % 8
        for vv in range(8):
            W2y[ww, vv * 16 + bj] = ka[vv] * COS[y, vv]
    mats["W2y"] = W2y
    # p2 C: [128 w, 64 wfc], wfc = v*8 + bj (W-pool folded)
    W2c = np.zeros((128, 64), dtype=np.float32)
    for ww in range(128):
        ss = ww // 2
        bj, y = ss // 8, ss % 8
        for vv in range(8):
            W2c[ww, vv * 8 + bj] = ka[vv] * COS[y, vv] * 0.5
    mats["W2c"] = W2c
    # quant tables in strip layout
    invTy = np.zeros((128, 128), dtype=np.float32)
    Ty = np.zeros((128, 128), dtype=np.float32)
    for hf in range(128):
        for wf in range(128):
            t = YT[hf // 16, wf // 16]
            Ty[hf, wf] = t
            invTy[hf, wf] = 1.0 / t
    mats["invTy"], mats["Ty"] = invTy, Ty
    invTc = np.zeros((128, 64), dtype=np.float32)
    Tc = np.zeros((128, 64), dtype=np.float32)
    for cf in range(128):
        uu = (cf % 64) // 8
        for wfc in range(64):
            t = CT[uu, wfc // 8]
            Tc[cf, wfc] = t
            invTc[cf, wfc] = 1.0 / t
    mats["invTc"], mats["Tc"] = invTc, Tc
    # p3 Y: [128 hf, 128 h]
    W3y = np.zeros((128, 128), dtype=np.float32)
    for hf in range(128):
        uu, bi = hf // 16, hf % 16
        for hh in range(128):
            if hh // 8 == bi:
                W3y[hf, hh] = ka[uu] * COS[hh % 8, uu]
    mats["W3y"] = W3y
    # p3 C block-diag: [128 (cc,uf), 256 (HCb|HCr)], H-upsample folded
    W3ch = np.zeros((64, 128), dtype=np.float32)
    for uf in range(64):
        uu, ub = uf // 8, uf % 8
        for hh in range(128):
            rr = hh // 2
            if rr // 8 == ub:
                W3ch[uf, hh] = ka[uu] * COS[rr % 8, uu]
    W3c = np.zeros((128, 256), dtype=np.float32)
    W3c[0:64, 0:128] = W3ch
    W3c[64:128, 128:256] = W3ch
    mats["W3c"] = W3c
    # p4 Y: [128 wf, 128 w] (with /255)
    W4y = np.zeros((128, 128), dtype=np.float32)
    for wf in range(128):
        vv, bj = wf // 16, wf % 16
        for ww in range(128):
            if ww // 8 == bj:
                W4y[wf, ww] = ka[vv] * COS[ww % 8, vv] / 255.0
    mats["W4y"] = W4y
    # p4 C base: [64 wfc, 128 w], W-upsample folded (with /255)
    W4c = np.zeros((64, 128), dtype=np.float32)
    for wfc in range(64):
        vv, bj = wfc // 8, wfc % 8
        for ww in range(128):
            ss = ww // 2
            if ss // 8 == bj:
                W4c[wfc, ww] = ka[vv] * COS[ss % 8, vv] / 255.0
    mats["W4cr_r"] = np.vstack([W4c * np.float32(Mi[0, 2]), np.zeros((64, 128), np.float32)])
    mats["W4cr_g"] = np.vstack([W4c * np.float32(Mi[1, 2]), np.zeros((64, 128), np.float32)])
    mats["W4cb_g"] = np.vstack([W4c * np.float32(Mi[1, 1]), np.zeros((64, 128), np.float32)])
    mats["W4cb_b"] = np.vstack([W4c * np.float32(Mi[2, 1]), np.zeros((64, 128), np.float32)])

    order = ["W1y0", "W1y1", "W1y2", "W1c0", "W1c1", "W1c2", "W2y", "W2c",
             "invTy", "Ty", "invTc", "Tc", "W3y", "W3c", "W4y",
             "W4cr_r", "W4cr_g", "W4cb_g", "W4cb_b"]
    cols = {}
    c0 = 0
    blocks = []
    for name in order:
        m = mats[name]
        assert m.shape[0] == 128, name
        cols[name] = (c0, m.shape[1])
        blocks.append(m.astype(np.float32))
        c0 += m.shape[1]
    CONST = np.concatenate(blocks, axis=1)
    return np.ascontiguousarray(CONST), cols


def _bcast(ap, n):
    """Repeat an SBUF AP n times along a new middle free dim (0-stride)."""
    return bass.AP(tensor=ap.tensor, offset=ap.offset,
                   ap=[ap.ap[0], [0, n]] + ap.ap[1:])


# DC bias for the Y path: blocks are (pixel-128) shifted; only the DC coef
# moves: coef00 -= 128 * 64 * 0.125 = 1024. Applied post-quant-scale.
_DC_BIAS_Q = -1024.0 / 6.4  # invTy[0,0] = 1/(16*0.4)


_NO_SPLIT = {"InstNoOp", "InstAllEngineBarrier", "InstEventSemaphore"}


def _split_pe_waits(nc):
    """HW engine instructions take a single sync wait; hoist extra waits onto
    preceding same-engine no-ops (ordering within an engine is preserved)."""
    for f in nc.m.functions:
        for bb in f.blocks:
            il = bb.instructions
            i = 0
            while i < len(il):
                ins = il[i]
                if type(ins).__name__ not in _NO_SPLIT:
                    si = getattr(ins, "sync_info", None)
                    if si is not None and si.on_wait and len(si.on_wait) > 1:
                        waits = list(si.on_wait)
                        for j, wt in enumerate(waits[:-1]):
                            nop = mybir.InstNoOp(name=f"{ins.name}-w{j}",
                                                 ins=[], outs=[])
                            nop.engine = ins.engine
                            nop.sync_info = mybir.SyncInfo(on_wait=[wt],
                                                           on_update=[])
                            try:
                                nc.register_instruction(nop, overwrite=True)
                            except Exception:
                                pass
                            il.insert(i, nop)
                            i += 1
                        ins.sync_info = mybir.SyncInfo(
                            on_wait=[waits[-1]],
                            on_update=list(si.on_update or []))
                i += 1


def _build_program(n_imgs=IMGS_PER_CORE, n_strips=NSTRIP, sim=False):
    CONST, cols = _build_consts()
    hh = n_strips * 128
    nc = bass.Bass("TRN2", target_bir_lowering=False) if sim else bass.Bass("TRN2")
    xin = nc.dram_tensor("xin", (n_imgs, 3, hh, W), F32, kind="ExternalInput")
    cin = nc.dram_tensor("consts", CONST.shape, F32, kind="ExternalInput")
    yout = nc.dram_tensor("yout", (n_imgs, 3, hh, W), F32, kind="ExternalOutput")

    with ExitStack() as ctx:
        tc = ctx.enter_context(tile.TileContext(nc))
        singles = ctx.enter_context(tc.tile_pool(name="singles", bufs=1))
        inpool = ctx.enter_context(tc.tile_pool(name="inpool", bufs=3))
        s1pool = ctx.enter_context(tc.tile_pool(name="s1pool", bufs=3))
        midpool = ctx.enter_context(tc.tile_pool(name="midpool", bufs=2))
        s3pool = ctx.enter_context(tc.tile_pool(name="s3pool", bufs=2))
        outpool = ctx.enter_context(tc.tile_pool(name="outpool", bufs=3))
        pp1 = ctx.enter_context(tc.tile_pool(name="pp1", bufs=2, space="PSUM"))
        pp2 = ctx.enter_context(tc.tile_pool(name="pp2", bufs=1, space="PSUM"))
        pp3y = ctx.enter_context(tc.tile_pool(name="pp3y", bufs=1, space="PSUM"))
        pp3c = ctx.enter_context(tc.tile_pool(name="pp3c", bufs=1, space="PSUM"))
        pp4 = ctx.enter_context(tc.tile_pool(name="pp4", bufs=2, space="PSUM"))

        ct = singles.tile([128, CONST.shape[1]], F32, tag="consts")
        nc.sync.dma_start(out=ct, in_=cin[:, :])
        # PE warm-up touch of the consts tile: makes PE observe the consts
        # DMA semaphore here, so no real matmul ever needs two sync waits
        # (HW LDWEIGHTS supports a single wait).
        warm = pp1.tile([128, 256], F32, tag="p1")
        nc.tensor.matmul(warm[0:1, 0:1], lhsT=ct[:, 0:1], rhs=ct[:, 0:1],
                         start=True, stop=True)

        def cview(name):
            c0, n = cols[name]
            return ct[:, c0:c0 + n]

        bias_out = singles.tile([128, 1], F32, tag="bias_out")
        nc.vector.memset(bias_out, 128.0 / 255.0)

        W1 = [(cview(f"W1y{ci}"), cview(f"W1c{ci}")) for ci in range(3)]
        W2y, W2c = cview("W2y"), cview("W2c")
        W3y, W3c = cview("W3y"), cview("W3c")
        W4y = cview("W4y")
        W4cr_r, W4cr_g = cview("W4cr_r")[0:64], cview("W4cr_g")[0:64]
        W4cb_g, W4cb_b = cview("W4cb_g")[0:64], cview("W4cb_b")[0:64]
        invTy, Ty = cview("invTy"), cview("Ty")
        invTc, Tc = cview("invTc"), cview("Tc")

        TT = nc.vector.tensor_tensor
        OP = mybir.AluOpType
        AF = mybir.ActivationFunctionType

        for img in range(n_imgs):
            for sp in range(n_strips):
                # ---- load strip [128 h, 3 c, 512 w]
                xt = inpool.tile([128, 3, W], F32, tag="instrip")
                src = xin[img, :, sp * 128:(sp + 1) * 128, :].rearrange("c h w -> h c w")
                nc.sync.dma_start(out=xt, in_=src)

                p2t = pp2.tile([128, 768], F32, tag="p2")
                for ch in range(NCH):
                    # ---- p1: one PSUM group, 6 matmuls (Y cols 0:128, C cols 128:256)
                    p1t = pp1.tile([128, 256], F32, tag="p1")
                    for ci in range(3):
                        lhs = xt[:, ci, ch * 128:(ch + 1) * 128]
                        nc.tensor.matmul(p1t[:, 0:128], lhsT=lhs, rhs=W1[ci][0],
                                         start=(ci == 0), stop=False)
                        nc.tensor.matmul(p1t[:, 128:256], lhsT=lhs, rhs=W1[ci][1],
                                         start=False, stop=(ci == 2))
                    s1t = s1pool.tile([128, 256], F32, tag="s1")
                    nc.scalar.activation(out=s1t, in_=p1t, func=AF.Copy)
                    # ---- p2: accumulate into strip-wide coef banks
                    nc.tensor.matmul(p2t[:, ch * 128:(ch + 1) * 128],
                                     lhsT=s1t[:, 0:128], rhs=W2y,
                                     start=(ch == 0), stop=(ch == NCH - 1))
                    nc.tensor.matmul(p2t[:, 512 + ch * 64:512 + (ch + 1) * 64],
                                     lhsT=s1t[:, 128:256], rhs=W2c,
                                     start=(ch == 0), stop=(ch == NCH - 1))

                # ---- middle (strip batch)
                qY = midpool.tile([128, 4, 128], F32, tag="qY")
                TT(out=qY, in0=p2t[:, 0:512].rearrange("p (g n) -> p g n", g=4),
                   in1=_bcast(invTy, 4), op=OP.mult)
                # Y DC bias: u=0 rows (0:16), v=0 cols (0:16 of each chunk)
                nc.vector.tensor_scalar_add(qY[0:16, :, 0:16], qY[0:16, :, 0:16],
                                            _DC_BIAS_Q)
                qC = midpool.tile([128, 4, 64], F32, tag="qC")
                TT(out=qC, in0=p2t[:, 512:768].rearrange("p (g n) -> p g n", g=4),
                   in1=_bcast(invTc, 4), op=OP.mult)

                # HW f32->i32 convert rounds to nearest even == jnp.round.
                # y = (r + (q-r)^3) * T.
                def middle(q, T_ap, nrep, tagp):
                    shp = [128, nrep, T_ap.shape[-1]]
                    rI = midpool.tile(shp, I32, tag=f"r{tagp}")
                    nc.vector.tensor_copy(out=rI, in_=q)
                    d = midpool.tile(shp, F32, tag=f"d{tagp}")
                    TT(out=d, in0=q, in1=rI, op=OP.subtract)
                    d2 = midpool.tile(shp, F32, tag=f"d2{tagp}")
                    nc.scalar.square(d2, d)
                    d3 = midpool.tile(shp, F32, tag=f"d3{tagp}")
                    TT(out=d3, in0=d2, in1=d, op=OP.mult)
                    s = midpool.tile(shp, F32, tag=f"s{tagp}")
                    TT(out=s, in0=rI, in1=d3, op=OP.add)
                    y = midpool.tile(shp, F32, tag=f"y{tagp}")
                    TT(out=y, in0=s, in1=_bcast(T_ap, nrep), op=OP.mult)
                    return y

                yY = middle(qY, Ty, 4, "Y")
                yC = middle(qC, Tc, 4, "C")

                # ---- p3
                p3yt = pp3y.tile([128, 512], F32, tag="p3y")
                for ch in range(NCH):
                    nc.tensor.matmul(p3yt[:, ch * 128:(ch + 1) * 128],
                                     lhsT=yY[:, ch, :], rhs=W3y,
                                     start=(ch == 0), stop=(ch == NCH - 1))
                s3y = s3pool.tile([128, 512], F32, tag="s3y")
                nc.scalar.activation(out=s3y, in_=p3yt, func=AF.Copy)
                s3c = s3pool.tile([64, 1024], F32, tag="s3c")
                for half in range(2):
                    p3ct = pp3c.tile([64, 512], F32, tag="p3c")
                    for k in range(2):
                        ch = half * 2 + k
                        nc.tensor.matmul(p3ct[:, k * 256:(k + 1) * 256],
                                         lhsT=yC[:, ch, :], rhs=W3c,
                                         start=(k == 0), stop=(k == 1))
                    nc.scalar.activation(out=s3c[:, half * 512:(half + 1) * 512],
                                         in_=p3ct, func=AF.Copy)

                # ---- p4 + output
                ot = outpool.tile([128, 3, W], F32, tag="outstrip")
                for ch in range(NCH):
                    p4t = pp4.tile([128, 384], F32, tag="p4")
                    ylhs = s3y[:, ch * 128:(ch + 1) * 128]
                    cblhs = s3c[:, ch * 256:ch * 256 + 128]
                    crlhs = s3c[:, ch * 256 + 128:ch * 256 + 256]
                    nc.tensor.matmul(p4t[:, 0:128], lhsT=ylhs, rhs=W4y, start=True, stop=False)
                    nc.tensor.matmul(p4t[:, 128:256], lhsT=ylhs, rhs=W4y, start=False, stop=False)
                    nc.tensor.matmul(p4t[:, 256:384], lhsT=ylhs, rhs=W4y, start=False, stop=False)
                    nc.tensor.matmul(p4t[:, 0:128], lhsT=crlhs, rhs=W4cr_r, start=False, stop=False)
                    nc.tensor.matmul(p4t[:, 128:256], lhsT=crlhs, rhs=W4cr_g, start=False, stop=False)
                    nc.tensor.matmul(p4t[:, 128:256], lhsT=cblhs, rhs=W4cb_g, start=False, stop=False)
                    nc.tensor.matmul(p4t[:, 256:384], lhsT=cblhs, rhs=W4cb_b, start=False, stop=True)
                    nc.scalar.activation(
                        out=ot[:, :, ch * 128:(ch + 1) * 128],
                        in_=p4t[:].rearrange("p (c w) -> p c w", c=3),
                        func=AF.Relu, bias=bias_out[:], scale=1.0)
                otf = ot[:].rearrange("p c w -> p (c w)")
                nc.vector.tensor_scalar_min(otf, otf, 1.0)
                dst = yout[img, :, sp * 128:(sp + 1) * 128, :].rearrange("c h w -> h c w")
                nc.sync.dma_start(out=dst, in_=ot)

    _split_pe_waits(nc)
    nc.finalize()
    return nc, CONST


_CACHE = {}


def kernel(x):
    x = np.ascontiguousarray(np.asarray(x, dtype=np.float32))
    assert x.shape == (64, 3, H, W)
    if "prog" not in _CACHE:
        _CACHE["prog"] = _build_program()
    nc, CONST = _CACHE["prog"]
    shards = x.reshape(N_CORES, IMGS_PER_CORE, 3, H, W)
    in_maps = [{"xin": shards[i], "consts": CONST} for i in range(N_CORES)]
    res = run_bass_kernel_spmd(nc, in_maps, core_ids=list(range(N_CORES)))
    out = np.stack([res.results[i]["yout"] for i in range(N_CORES)])
    return out.reshape(64, 3, H, W)


if __name__ == "__main__":
    rng = np.random.RandomState(0)
    x = rng.rand(64, 3, H, W).astype(np.float32)
    y = kernel(x)
    print("out", y.shape, y.dtype, y.min(), y.max())
